# revision 1
# baseline (speedup 1.0000x reference)
"""Trainium2 Bass kernel for nn_Biaffine (B=4, S=512, D=512, R=64).

Math: the reference computes
    left = einsum('bxi,irj,byj->bxyr', hf, U1, hb)
    out  = mean_y(left + rf[:, :, None] + rb[:, None] + bias)
The mean over y commutes with everything:
    mean_y(left)[b,x,r] = sum_ij hf[b,x,i] U1[i,r,j] hbbar[b,j],
    hbbar = mean_y(hb).
So out[b,x,r] = sum_i hf[b,x,i] * (V[b,i,r] + U2a[i,r]) + rbbar[b,r] + bias[r]
with V[b,i,r] = sum_j U1[i,r,j] hbbar[b,j], rbbar = hbbar @ U2b.

Sharding: tensor-parallel over r (dep_vec_dim): core c owns r in [8c, 8c+8).
Each core reads its U1 shard (8.4MB, the dominant traffic) and full hf (4MB);
hb's mean is y-sharded (512KB/core) and combined with an 8KB on-chip AllReduce.
Each core computes out[:, :, 8c:8c+8]; the host concatenates. Per-core HBM
traffic ~12.9MB at the measured ~180GB/s/core puts the DMA floor at ~72us.
"""

import os
import sys

import numpy as np

try:
    import concourse.bass as bass  # noqa: F401
except ImportError:  # pragma: no cover
    sys.path.insert(0, "/opt/trn_rl_repo")

B, S, D, R = 4, 512, 512, 64
NCORES = 8
RB = R // NCORES  # 8 r's per core
P = 128
JC = D // P  # 4 j-chunks
IC = D // P  # 4 i-chunks
SY = S // NCORES  # 64 y's per core (mean partial, AllReduce'd)

# module-level knobs / results (test.py uses these; harness doesn't need them)
TRACE = os.environ.get("BASS_KERNEL_TRACE", "0") == "1"
LAST_RESULTS = None

_NC_CACHE = {}


def _build_nc(n_repeat=1, solo_ar=False):
    import concourse.bacc as bacc
    import concourse.mybir as mybir
    import concourse.tile as tile
    from concourse.masks import make_identity
    fp32 = mybir.dt.float32

    nc = bacc.Bacc("TRN2", target_bir_lowering=False, debug=False, num_devices=NCORES)

    hft_d = nc.dram_tensor("hft", [B, D, S], fp32, kind="ExternalInput")
    hb_d = nc.dram_tensor("hb", [D, B, SY], fp32, kind="ExternalInput")
    u1t_d = nc.dram_tensor("u1t", [D, RB, D], fp32, kind="ExternalInput")
    u2t_d = nc.dram_tensor("u2t", [P, IC, 2 * RB], fp32, kind="ExternalInput")
    bias_d = nc.dram_tensor("biasr", [1, RB], fp32, kind="ExternalInput")
    out_d = nc.dram_tensor("out", [B, RB, S], fp32, kind="ExternalOutput")

    with tile.TileContext(nc) as tc:
        with (
            tc.tile_pool(name="const", bufs=1) as cpool,
            tc.tile_pool(name="data", bufs=1) as dpool,
            tc.tile_pool(name="psum", bufs=8, space="PSUM") as ppool,
            tc.tile_pool(name="dram", bufs=1, space="DRAM") as drpool,
        ):
            identity_sq = cpool.tile([100, 100], fp32, tag="identity_sq")
            make_identity(nc, identity_sq)
            ones1 = cpool.tile([1, S], fp32, tag="ones1")
            nc.vector.memset(ones1, 1.0)

            for _rep in range(n_repeat):
                _emit_body(
                    nc, dpool, ppool, drpool, fp32, ones1, identity_sq,
                    hft_d, hb_d, u1t_d, u2t_d, bias_d, out_d, solo_ar,
                )

    nc.compile()
    return nc


def _emit_body(
    nc, dpool, ppool, drpool, fp32, ones1, identity_sq,
    hft_d, hb_d, u1t_d, u2t_d, bias_d, out_d, solo_ar=False,
):
    import concourse.mybir as mybir
    if True:
        if True:
            u2sb = dpool.tile([P, IC, 2 * RB], fp32, tag="u2sb", bufs=2)
            bias_sb = dpool.tile([1, RB], fp32, tag="bias_sb", bufs=2)
            hbbarT = dpool.tile([P, JC * B], fp32, tag="hbbarT", bufs=2)
            rbb = dpool.tile([B, RB], fp32, tag="rbb", bufs=2)
            vass = dpool.tile([P, IC, B, RB], fp32, tag="vass", bufs=2)

            # --- small inputs (u2sb arrives host-pre-packed as [d%P, dchunk, 2*RB]) ---
            nc.sync.dma_start(out=u2sb, in_=u2t_d.ap())
            nc.sync.dma_start(out=bias_sb, in_=bias_d.ap())

            # --- hb y-slice load, host-transposed to [j, b, y] so the partial
            # mean is a DVE free-axis reduce (no PE matmuls, no PSUM trip);
            # the 1/S factor is folded into the host-side U1/U2b scaling
            hbt = dpool.tile([P, JC, B, SY], fp32, tag="hb", bufs=2)
            nc.sync.dma_start(
                out=hbt, in_=hb_d.ap().rearrange("(jc p) b y -> p jc b y", p=P)
            )

            # --- big loads issued up-front: the SP DGE queue is in-order, so
            # no DMA with a semaphore wait may precede these (head-of-line).
            # u1 bufs=1 is safe: V(i) finishes before hft(i) drains, so the
            # slot-free wait for u1t(i+1) never starves the rings.
            u1_tiles = []
            for jc in range(JC):
                u1t_t = dpool.tile([P, RB, D], fp32, tag=f"u1_{jc}")
                nc.sync.dma_start(out=u1t_t, in_=u1t_d.ap()[jc * P : (jc + 1) * P])
                u1_tiles.append(u1t_t)
            hft_tiles = []
            for b in range(B):
                hft_t = dpool.tile([P, IC, S], fp32, tag=f"hft{b}", bufs=2)
                nc.sync.dma_start(
                    out=hft_t, in_=hft_d.ap()[b].rearrange("(ic p) x -> p ic x", p=P)
                )
                hft_tiles.append(hft_t)

            # partial hbbarT[j, b] = sum_{y in slice} hb[b, y, j] (unscaled;
            # U1T/U2b carry the 1/S), via DVE free-axis reduces
            hbbarT_part = dpool.tile([P, JC * B], fp32, tag="hbbarT_part", bufs=2)
            for b in range(B):
                for jc in range(JC):
                    nc.vector.reduce_sum(
                        hbbarT_part[:, jc * B + b : jc * B + b + 1],
                        hbt[:, jc, b, :],
                        axis=mybir.AxisListType.X,
                    )

            # --- AllReduce the 8KB partial means across the 8 cores ---
            ar_in = drpool.tile([P, JC * B], fp32, tag="ar_in")
            ar_out = drpool.tile([P, JC * B], fp32, tag="ar_out")
            nc.scalar.dma_start(out=ar_in[:], in_=hbbarT_part)
            nc.gpsimd.collective_compute(
                "AllReduce",
                mybir.AluOpType.add,
                replica_groups=(
                    [[c] for c in range(NCORES)] if solo_ar
                    else [list(range(NCORES))]
                ),
                ins=[ar_in.opt()],
                outs=[ar_out.opt()],
            )
            nc.scalar.dma_start(out=hbbarT, in_=ar_out[:])

            # --- rbbar[b, r] = hbbar @ U2b (+ bias via K=1 ones-matmul) ---
            ps_rb = ppool.tile([P, 512], fp32, tag="ps")
            for jc in range(JC):
                nc.tensor.matmul(
                    ps_rb[:B, :RB],
                    hbbarT[:, jc * B : (jc + 1) * B],
                    u2sb[:, jc, RB : 2 * RB],
                    start=(jc == 0),
                    stop=False,
                )
            nc.tensor.matmul(
                ps_rb[:B, :RB], ones1[:1, :B], bias_sb, start=False, stop=True
            )
            nc.vector.tensor_copy(out=rbb, in_=ps_rb[:B, :RB])
            # transpose to [r, b] so (rbbar+bias) can be added to the output
            # tiles as a per-partition broadcast during the PSUM->SBUF copy
            ps_rbt = ppool.tile([P, 512], fp32, tag="ps")
            nc.tensor.transpose(ps_rbt[:RB, :B], rbb, identity_sq[:B, :B])
            rbbT = dpool.tile([RB, B], fp32, tag="rbbT", bufs=2)
            nc.vector.tensor_copy(out=rbbT, in_=ps_rbt[:RB, :B])

            # --- V[b, i] per r: hbbarT stationary (LDW = 4 cols), U1 streams
            # as the N=512 moving operand. Four r's share one PSUM tile at
            # base partitions {0,32,64,96} (legal tile_position[1] for M=4),
            # so the [b, i] -> [i, b] PE transposes drop from 32 to 8.
            for rq in range(RB // 4):
                ps_q = ppool.tile([P, 512], fp32, tag="ps")
                for k in range(4):
                    r = rq * 4 + k
                    for jc in range(JC):
                        nc.tensor.matmul(
                            ps_q[k * 32 : k * 32 + B, :D],
                            hbbarT[:, jc * B : (jc + 1) * B],
                            u1_tiles[jc][:, r, :],
                            start=(jc == 0),
                            stop=(jc == JC - 1),
                            tile_position=(0, k * 32),
                        )
                vq = dpool.tile([100, D], fp32, tag="vq", bufs=2)
                nc.vector.tensor_copy(out=vq, in_=ps_q[:100, :D])
                for ic in range(IC):
                    ps_t = ppool.tile([P, 512], fp32, tag="ps")
                    nc.tensor.transpose(
                        ps_t[:P, :100], vq[:, ic * P : (ic + 1) * P], identity_sq
                    )
                    # one strided add moves all 4 r's: ps_t cols (k*32 + b),
                    # viewed [p, k, b] -> [p, b, k], into vass[:, ic, b, r]
                    nc.vector.tensor_tensor(
                        out=vass[:, ic, :, rq * 4 : (rq + 1) * 4],
                        in0=ps_t[:, :128]
                        .rearrange("p (k c) -> p k c", c=32)[:, :, :B]
                        .rearrange("p k b -> p b k"),
                        in1=u2sb[:, ic, None, rq * 4 : (rq + 1) * 4].to_broadcast(
                            (P, B, 4)
                        ),
                        op=mybir.AluOpType.add,
                    )

            # --- out[r, x] per b: contract i; K=1 augment adds rbbar+bias ---
            for b in range(B):
                ps_o = ppool.tile([P, 512], fp32, tag="ps")
                for ic in range(IC):
                    nc.tensor.matmul(
                        ps_o[:RB, :S],
                        vass[:, ic, b, :],
                        hft_tiles[b][:, ic, :],
                        start=(ic == 0),
                        stop=(ic == IC - 1),
                    )
                out_sb_b = dpool.tile([RB, S], fp32, tag=f"out{b}", bufs=2)
                nc.vector.tensor_tensor(
                    out=out_sb_b,
                    in0=ps_o[:RB, :S],
                    in1=rbbT[:, b : b + 1].to_broadcast((RB, S)),
                    op=mybir.AluOpType.add,
                )
                nc.scalar.dma_start(out=out_d.ap()[b], in_=out_sb_b)


def _get_nc(n_repeat=1):
    if n_repeat not in _NC_CACHE:
        _NC_CACHE[n_repeat] = _build_nc(n_repeat)
    return _NC_CACHE[n_repeat]


def _prep_inputs(h_forward, h_backward, U_1, U_2, bias):
    hf = np.ascontiguousarray(np.asarray(h_forward, dtype=np.float32))
    hb = np.ascontiguousarray(np.asarray(h_backward, dtype=np.float32))
    u1 = np.asarray(U_1, dtype=np.float32)
    u2 = np.asarray(U_2, dtype=np.float32)
    bz = np.asarray(bias, dtype=np.float32)

    hft = np.ascontiguousarray(hf.transpose(0, 2, 1))  # [B, i, x]

    in_maps = []
    for c in range(NCORES):
        rs = slice(c * RB, (c + 1) * RB)
        # 1/S premultiplied: hbbar arrives as a plain sum over y
        u1t_c = np.ascontiguousarray(
            u1[:, rs, :].transpose(2, 1, 0) * np.float32(1.0 / S)
        )  # [j, r, i]
        # pre-packed u2sb layout [d%P, dchunk, 2*RB]: cols 0:RB = U2a[d, rs],
        # RB:2RB = U2b[d, rs]
        u2t_c = np.ascontiguousarray(
            np.concatenate(
                [
                    u2[:D, rs].reshape(IC, P, RB).transpose(1, 0, 2),
                    u2[D:, rs].reshape(IC, P, RB).transpose(1, 0, 2)
                    * np.float32(1.0 / S),
                ],
                axis=2,
            )
        )
        bias_c = np.ascontiguousarray(bz[rs].reshape(1, RB))
        hb_c = np.ascontiguousarray(
            hb[:, c * SY : (c + 1) * SY, :].transpose(2, 0, 1)
        )  # [D(j), B, SY]
        in_maps.append(
            {
                "hft": hft,
                "hb": hb_c,
                "u1t": u1t_c,
                "u2t": u2t_c,
                "biasr": bias_c,
            }
        )
    return in_maps


def _get_exec():
    """One jitted sharded executable, cached for the process lifetime.

    Repeated kernel() calls reuse it — re-jitting a second executable with
    collectives in the same process has been observed to wedge the NRT
    (NRT_EXEC_UNIT_UNRECOVERABLE), while re-executing one executable is solid.
    """
    if "exec" in _EXEC_CACHE:
        return _EXEC_CACHE["exec"]

    import jax
    from jax.sharding import Mesh, PartitionSpec

    import warnings

    with warnings.catch_warnings():
        warnings.simplefilter("ignore")
        from jax.experimental.shard_map import shard_map

    from concourse import mybir
    from concourse.bass2jax import (
        _bass_exec_p,
        install_neuronx_cc_hook,
        partition_id_tensor,
    )

    install_neuronx_cc_hook()
    nc = _get_nc()
    partition_name = nc.partition_id_tensor.name if nc.partition_id_tensor else None
    in_names, out_names, out_avals = [], [], []
    for alloc in nc.m.functions[0].allocations:
        if not isinstance(alloc, mybir.MemoryLocationSet):
            continue
        name = alloc.memorylocations[0].name
        if alloc.kind == "ExternalInput":
            if name != partition_name:
                in_names.append(name)
        elif alloc.kind == "ExternalOutput":
            out_names.append(name)
            out_avals.append(
                jax.core.ShapedArray(tuple(alloc.tensor_shape), mybir.dt.np(alloc.dtype))
            )
    all_names = in_names + out_names
    if partition_name is not None:
        all_names = all_names + [partition_name]

    def _body(*args):
        operands = list(args)
        if partition_name is not None:
            operands.append(partition_id_tensor())
        return tuple(
            _bass_exec_p.bind(
                *operands,
                out_avals=tuple(out_avals),
                in_names=tuple(all_names),
                out_names=tuple(out_names),
                lowering_input_output_aliases=(),
                sim_require_finite=True,
                sim_require_nnan=True,
                nc=nc,
            )
        )

    devices = jax.devices()[:NCORES]
    mesh = Mesh(np.asarray(devices), ("core",))
    n_args = len(in_names) + len(out_avals)
    fn = jax.jit(
        shard_map(
            _body,
            mesh=mesh,
            in_specs=(PartitionSpec("core"),) * n_args,
            out_specs=(PartitionSpec("core"),) * len(out_names),
            check_rep=False,
        ),
        keep_unused=True,
    )
    sh = jax.sharding.NamedSharding(mesh, PartitionSpec("core"))
    _EXEC_CACHE["exec"] = (fn, sh, in_names, out_names, out_avals)
    return _EXEC_CACHE["exec"]


_EXEC_CACHE = {}


def kernel(h_forward, h_backward, U_1, U_2, bias):
    import jax

    fn, sh, in_names, out_names, out_avals = _get_exec()
    in_maps = _prep_inputs(h_forward, h_backward, U_1, U_2, bias)
    args = [
        jax.device_put(
            np.concatenate([in_maps[c][name] for c in range(NCORES)], axis=0), sh
        )
        for name in in_names
    ]
    for av in out_avals:
        args.append(
            jax.device_put(
                np.zeros((NCORES * av.shape[0], *av.shape[1:]), av.dtype), sh
            )
        )
    out_arrs = fn(*args)
    oi = out_names.index("out")
    full = np.asarray(out_arrs[oi]).reshape(NCORES, B, RB, S)  # [core, B, RB, S]
    out = np.concatenate(list(full), axis=1)  # [B, R, S]
    return np.ascontiguousarray(out.transpose(0, 2, 1))  # [B, S, R]

